# revision 35
# baseline (speedup 1.0000x reference)
"""MoE ConvNeXt block (dwconv7x7 -> LN -> top2-of-8 MoE MLP -> layerscale residual)
on 8 trn2 NeuronCores, data-parallel over batch (4 images / 4096 tokens per core).

This version implements TRUE top-2 routed dispatch on-device (capacity-factor
routing, C=1024 slots/expert = 2x the per-expert mean load of T*2/8=1024 -> CF 1.0),
replacing the dense 8-expert baseline. Token compaction is done with gpsimd ISA ops
(probed working on this hardware: see probe1.py / probe2.py):

 - dwconv 7x7 + LN: diagonal-stationary fp8 DoubleRow matmuls + ones-matmul stats
   (as the dense baseline); LN output written channel-major bf16, chunks (0,1)
   interleaved per token so a token's (c0,c1) bf16 pair bitcasts to one f32 for
   4-byte-granular ap_gather.
 - router: bf16 x tiles as matmul stationaries -> top-2 + softmax via DVE.
 - compaction: per expert, two parallel arrays (token-id, gate-weight/16) with -1
   at unrouted positions, laid out in gpsimd wrapped order (logical i = free*16 +
   partition = token id) -> sparse_gather compacts both in token order and pads
   tails with -1 (=> empty slots get gate weight exactly 0 after clamping).
 - ranks (token -> slot for the gather-back): strict-lower-triangular ones matmuls
   give intra-tile prefix counts; 32x32 DVE block transposes + a second triangular
   matmul give inter-tile offsets; slot = base_e + offset + intra, selected per
   token by its top-2 expert masks.
 - expert MLP: 16 blocks of 512 compacted slots, weight-stationary fp8 DoubleRow
   (contraction padded 384->512: pair0=(c0,c1), pair1=(c2, bias-row chunk where
   partition 0 of the padded x chunk is 1/16 so W1's bias row folds b1 in); gelu
   on ScalarE over 2-bank 1024-wide PSUM tiles; the per-slot gate weight (and b2)
   is fused into the PSUM->bf16 copy: Y = (psum + 16*b2) * (w/16).
 - gather-back: Y stored as (c0,c1)- and (c2,x)-interleaved bf16 pairs, gathered
   per token with interleaved (slot0,slot1) indices in one ap_gather per pair
   array; combine = lane-sum + layer_scale*(.) + residual, straight to output.

Capacity overflow (per-expert count > 1024, ~9 sigma away from the binomial mean
under this router distribution) drops the overflowing expert's contribution for
those tokens and clamps their slot into the region (bounded ~1e-7 absolute noise
vs layer_scale=1e-6 branch scale). fp8/bf16 quantization dominates the error
budget either way; measured end-to-end rel err ~1e-7.
"""

import sys

sys.path.insert(0, "/opt/trn_rl_repo/concourse")
sys.path.insert(0, "/opt/trn_rl_repo")

import numpy as np
import ml_dtypes

import concourse.bass as bass
import concourse.tile as tile
from concourse import bacc, mybir
from concourse import bass_utils

F32 = mybir.dt.float32
BF16 = mybir.dt.bfloat16
FP8 = mybir.dt.float8e4
I16 = mybir.dt.int16
U32 = mybir.dt.uint32
AF = mybir.ActivationFunctionType
OP = mybir.AluOpType

DIM = 384
NE = 8
HID = 4 * DIM  # 1536
NIMG = 4  # images per core
T = NIMG * 1024  # tokens per core
NQ = 3  # channel chunks of 128
NHT = HID // 128  # 12
NCB = 8  # 512-token column blocks
CB = 512
CAP = 1024  # capacity per expert (slots)
S = NE * CAP  # 8192 total slots
NBLK = S // CB  # 16 MLP blocks
EPS = 1e-6

_cached = None


def _build():
    nc = bacc.Bacc("TRN2", target_bir_lowering=False)

    inp4 = nc.dram_tensor("inp4", [NIMG, DIM, 32, 32], F32, kind="ExternalInput")
    dgp = nc.dram_tensor("dgp", [NQ, 7, 4, 128, 2, 128], FP8, kind="ExternalInput")
    inp8 = nc.dram_tensor("inp8", [NQ, 128, 2, NIMG, 38, 38], FP8, kind="ExternalInput")
    w1q = nc.dram_tensor("w1q", [NE, 2, 128, 2, HID], FP8, kind="ExternalInput")
    w2p = nc.dram_tensor("w2p", [NE, 6, 128, 2, DIM], FP8, kind="ExternalInput")
    b2s = nc.dram_tensor("b2s", [128, NE, NQ], F32, kind="ExternalInput")
    gws = nc.dram_tensor("gws", [128, NQ, NE], BF16, kind="ExternalInput")
    chv = nc.dram_tensor("chv", [128, NQ, 4], F32, kind="ExternalInput")
    io8 = nc.dram_tensor("io8", [128, NE], F32, kind="ExternalInput")
    tid = nc.dram_tensor("tid", [128, 32], F32, kind="ExternalInput")
    ltr = nc.dram_tensor("ltr", [128, 128], BF16, kind="ExternalInput")
    out4 = nc.dram_tensor("out4", [NIMG, DIM, 32, 32], F32, kind="ExternalOutput")

    inp_cm = inp4.rearrange("n c h w -> c n (h w)")  # [384, 4, 1024]
    out_cm = out4.rearrange("n c h w -> c n (h w)")

    with tile.TileContext(nc) as tc:
        # ---------- persistent SBUF ----------
        persist = tc.alloc_tile_pool(name="persist", bufs=1)
        b2t = persist.tile([128, NE, NQ], F32, tag="b2t", name="b2t")
        gwt = persist.tile([128, NQ, NE], BF16, tag="gwt", name="gwt")
        chvt = persist.tile([128, NQ, 4], F32, tag="chvt", name="chvt")
        io8t = persist.tile([128, NE], F32, tag="io8t", name="io8t")
        tidt = persist.tile([128, 32], F32, tag="tidt", name="tidt")
        ltrit = persist.tile([128, 128], BF16, tag="ltrit", name="ltrit")
        onest = persist.tile([128, 128], BF16, tag="onest", name="onest")
        m1v = persist.tile([128, 32], F32, tag="m1v", name="m1v")
        m2v = persist.tile([128, 32], F32, tag="m2v", name="m2v")
        # router outputs packed in one tile: [e0, e1, w0, w1] -> one DMA bounce
        rt4 = persist.tile([128, 4, 32], F32, tag="rt4", name="rt4")
        def e0v(*s):
            return rt4[:, 0, :] if not s else rt4[:, 0, s[0]]
        def e1v(*s):
            return rt4[:, 1, :] if not s else rt4[:, 1, s[0]]
        def w0v(*s):
            return rt4[:, 2, :] if not s else rt4[:, 2, s[0]]
        def w1v(*s):
            return rt4[:, 3, :] if not s else rt4[:, 3, s[0]]

        nc.sync.dma_start(b2t[:], b2s[:])
        nc.sync.dma_start(gwt[:], gws[:])
        nc.sync.dma_start(chvt[:], chv[:])
        nc.sync.dma_start(io8t[:], io8[:])
        nc.sync.dma_start(tidt[:], tid[:])
        nc.sync.dma_start(ltrit[:], ltr[:])
        nc.any.memset(onest[:], 1.0)
        epst = persist.tile([128, 1], F32, tag="epst", name="epst")
        nc.any.memset(epst[:], EPS)
        zerot = persist.tile([128, 1], F32, tag="zerot", name="zerot")
        nc.any.memset(zerot[:], 0.0)

        # LN output, chunks (0,1) interleaved + (2, zero) interleaved
        # (own pool: released after gather-in so the MLP phase can reuse the space)
        xpool = tc.alloc_tile_pool(name="xpool", bufs=1)
        xbi = xpool.tile([128, T, 2], BF16, tag="xbi", name="xbi")
        xb2 = xpool.tile([128, T, 2], BF16, tag="xb2", name="xb2")
        nc.gpsimd.memset(xb2[:, :, 1], 0.0)

        # ---------- phase 1: dwconv ----------
        with tc.tile_pool(name="convin", bufs=3) as cpool, \
             tc.tile_pool(name="diagp", bufs=3) as dpool, \
             tc.tile_pool(name="xconv", bufs=1) as xcpool, \
             tc.tile_pool(name="cps", bufs=6, space="PSUM") as cps, \
             tc.tile_pool(name="sps", bufs=1, space="PSUM") as sps, \
             tc.tile_pool(name="lnt", bufs=2) as lnt:
            xconv = [xcpool.tile([128, T], BF16, tag=f"xc{q}", name=f"xc{q}") for q in range(NQ)]
            xp8s, dgpts = [], []
            for q in range(NQ):
                xp8 = cpool.tile([128, 2, NIMG, 38, 38], FP8, tag="xp8", name="xp8")
                nc.sync.dma_start(xp8[:], inp8[q])
                xp8s.append(xp8)
                dgpt = dpool.tile([128, 7, 4, 2, 128], FP8, tag="dgpt", name="dgpt")
                nc.sync.dma_start(dgpt[:], dgp.rearrange("q w j p t m -> p q w j t m")[:, q])
                dgpts.append(dgpt)
            for cbg in range(2):
                for q in range(NQ):
                    xp8, dgpt = xp8s[q], dgpts[q]
                    pts = [cps.tile([128, 16, 32], F32, tag="cpsum", name="cpsum") for _ in range(4)]
                    for dw in range(7):
                        for jp in range(4):
                            for j in range(4):
                                cb = cbg * 4 + j
                                n, hh = cb // 2, cb % 2
                                a = hh * 16 + 2 * jp
                                nc.tensor.matmul(
                                    pts[j][:],
                                    dgpt[:, dw, jp],
                                    xp8[:, :, n, a: a + 16, dw: dw + 32],
                                    start=(dw == 0 and jp == 0),
                                    stop=(dw == 6 and jp == 3),
                                    perf_mode=mybir.MatmulPerfMode.DoubleRow,
                                )
                    for j in range(4):
                        cb = cbg * 4 + j
                        sl = slice(cb * CB, (cb + 1) * CB)
                        xcv = xconv[q][:, sl].rearrange("p (a b) -> p a b", a=16)
                        nc.scalar.activation(xcv, pts[j][:], AF.Identity,
                                             bias=chvt[:, q, 0:1], scale=1.0 / 16.0)

                # ---------- LN stats + apply for this column group ----------
                for cb in range(cbg * 4, cbg * 4 + 4):
                    sl = slice(cb * CB, (cb + 1) * CB)
                    pm1 = sps.tile([128, CB], F32, tag="pm1", name="pm1")
                    pm2 = sps.tile([128, CB], F32, tag="pm2", name="pm2")
                    for q in range(NQ):
                        nc.tensor.matmul(pm1[:], onest[:], xconv[q][:, sl],
                                         start=(q == 0), stop=(q == NQ - 1))
                    for q in range(NQ):
                        sqt = lnt.tile([128, CB], BF16, tag="sqt", name="sqt")
                        nc.scalar.activation(sqt[:], xconv[q][:, sl], AF.Square,
                                             bias=zerot[:], scale=1.0)
                        nc.tensor.matmul(pm2[:], onest[:], sqt[:],
                                         start=(q == 0), stop=(q == NQ - 1))
                    mus = lnt.tile([128, CB], F32, tag="mus", name="mus")
                    nc.scalar.activation(mus[:], pm1[:], AF.Identity,
                                         bias=zerot[:], scale=1.0 / DIM)
                    msq = lnt.tile([128, CB], F32, tag="msq", name="msq")
                    nc.scalar.activation(msq[:], mus[:], AF.Square,
                                         bias=zerot[:], scale=1.0)
                    var = lnt.tile([128, CB], F32, tag="var", name="var")
                    nc.gpsimd.scalar_tensor_tensor(var[:], pm2[:], 1.0 / DIM, msq[:],
                                                   OP.mult, OP.subtract)
                    sd = lnt.tile([128, CB], F32, tag="sd", name="sd")
                    nc.scalar.activation(sd[:], var[:], AF.Sqrt, bias=epst[:], scale=1.0)
                    rst = lnt.tile([128, CB], F32, tag="rst", name="rst")
                    nc.vector.reciprocal(rst[:], sd[:])
                    for q in range(NQ):
                        t1 = lnt.tile([128, CB], F32, tag="t1", name="t1")
                        nc.vector.tensor_tensor(t1[:], xconv[q][:, sl], mus[:],
                                                OP.subtract)
                        t2 = lnt.tile([128, CB], F32, tag="t2", name="t2")
                        nc.vector.tensor_tensor(t2[:], t1[:], rst[:], OP.mult)
                        dst = xbi[:, sl, q] if q < 2 else xb2[:, sl, 0]
                        nc.scalar.activation(dst, t2[:], AF.Identity,
                                             bias=chvt[:, q, 2:3], scale=chvt[:, q, 1:2])

        # ---------- phase 3: router logits + top-2 ----------
        with tc.tile_pool(name="lps", bufs=4, space="PSUM") as lps, \
             tc.tile_pool(name="tkt", bufs=6) as tkt:
            for tt in range(32):
                plg = lps.tile([128, NE], F32, tag="plg", name="plg")
                tsl = slice(tt * 128, (tt + 1) * 128)
                for q in range(NQ):
                    xs = xbi[:, tsl, q] if q < 2 else xb2[:, tsl, 0]
                    nc.tensor.matmul(plg[:], xs, gwt[:, q],
                                     start=(q == 0), stop=(q == NQ - 1))
                c1 = slice(tt, tt + 1)
                nc.vector.tensor_reduce(m1v[:, c1], plg[:], mybir.AxisListType.X, OP.max)
                ta = tkt.tile([128, NE], F32, tag="ta", name="ta")
                nc.gpsimd.tensor_scalar(ta[:], plg[:], m1v[:, c1], None, OP.is_equal)
                tb = tkt.tile([128, NE], F32, tag="tb", name="tb")
                nc.gpsimd.tensor_tensor(tb[:], ta[:], io8t[:], OP.mult)
                nc.vector.tensor_reduce(e0v(c1), tb[:], mybir.AxisListType.X, OP.max)
                tcm = tkt.tile([128, NE], F32, tag="tc", name="tc")
                nc.gpsimd.scalar_tensor_tensor(tcm[:], ta[:], -1e30, plg[:],
                                               OP.mult, OP.add)
                nc.vector.tensor_reduce(m2v[:, c1], tcm[:], mybir.AxisListType.X, OP.max)
                td = tkt.tile([128, NE], F32, tag="td", name="td")
                nc.gpsimd.tensor_scalar(td[:], tcm[:], m2v[:, c1], None, OP.is_equal)
                te = tkt.tile([128, NE], F32, tag="te", name="te")
                nc.gpsimd.tensor_tensor(te[:], td[:], io8t[:], OP.mult)
                nc.vector.tensor_reduce(e1v(c1), te[:], mybir.AxisListType.X, OP.max)
            # softmax over the two top values (w0 for e0, w1 for e1)
            dv = tkt.tile([128, 32], F32, tag="dv", name="dv")
            nc.vector.tensor_tensor(dv[:], m2v[:], m1v[:], OP.subtract)
            ev = tkt.tile([128, 32], F32, tag="ev", name="ev")
            nc.scalar.activation(ev[:], dv[:], AF.Exp, bias=zerot[:], scale=1.0)
            den = tkt.tile([128, 32], F32, tag="den", name="den")
            nc.vector.tensor_scalar_add(den[:], ev[:], 1.0)
            nc.vector.reciprocal(w0v(), den[:])
            nc.vector.tensor_scalar(w1v(), w0v(), -1.0, 1.0, OP.mult, OP.add)

        xg8pool = tc.alloc_tile_pool(name="xg8pool", bufs=1, side="right")
        xg8 = xg8pool.tile([128, 4, S], FP8, tag="xg8", name="xg8")
        wslbc = xg8pool.tile([128, S], BF16, tag="wslbc", name="wslbc")
        nc.gpsimd.memset(xg8[:, 3, :], 0.0)
        nc.any.memset(xg8[0:1, 3, :], 1.0 / 16.0)  # bias row for W1's b1 fold

        # ---------- phase 4: routing tables (lists, ranks, slots) ----------
        dram = tc.alloc_tile_pool(name="dramscratch", bufs=1, space="DRAM")
        sl01 = dram.tile([2, T], F32, name="sl01")
        w8dr = dram.tile([16, 512], BF16, name="w8dr")
        odr = dram.tile([NE, 32], F32, name="odr")

        idxit = persist.tile([128, 512], I16, tag="idxit", name="idxit")
        idxot = persist.tile([128, 512], I16, tag="idxot", name="idxot")

        rtt = tc.alloc_tile_pool(name="rtt", bufs=1)
        rps = tc.alloc_tile_pool(name="rps", bufs=2, space="PSUM")
        if True:
            m0s = rtt.tile([128, NE, 32], F32, tag="m0s", name="m0s")
            m1s = rtt.tile([128, NE, 32], F32, tag="m1s", name="m1s")
            mbf = rtt.tile([128, NE, 32], BF16, tag="mbf", name="mbf")
            vind = rtt.tile([128, NE, 32], F32, tag="vind", name="vind")
            wind = rtt.tile([128, NE, 32], F32, tag="wind", name="wind")
            tidp1 = rtt.tile([128, 32], F32, tag="tidp1", name="tidp1")
            nc.vector.tensor_scalar_add(tidp1[:], tidt[:], 1.0)
            msumf = rtt.tile([128, NE, 32], F32, tag="msumf", name="msumf")
            nc.vector.memset(vind[:], -1.0)
            nc.vector.memset(wind[:], -1.0)
            for e in range(NE):
                ve = nc.vector if e < 4 else nc.gpsimd
                ve.tensor_scalar(m0s[:, e], e0v(), float(e), None, OP.is_equal)
                ve.tensor_scalar(m1s[:, e], e1v(), float(e), None, OP.is_equal)
                ve.tensor_tensor(msumf[:, e], m0s[:, e], m1s[:, e], OP.add)
                nc.vector.tensor_scalar_add(mbf[:, e], msumf[:, e], 0.0)
                # token-id array: t where routed, else -1
                nc.vector.copy_predicated(vind[:, e], msumf[:, e], tidt[:])
                # weight array: w/16 where routed, else -1
                wa = rtt.tile([128, 32], F32, tag="wa", name="wa")
                ve.tensor_tensor(wa[:], m0s[:, e], w0v(), OP.mult)
                wb = rtt.tile([128, 32], F32, tag="wb", name="wb")
                ve.tensor_tensor(wb[:], m1s[:, e], w1v(), OP.mult)
                wc = rtt.tile([128, 32], F32, tag="wc", name="wc")
                ve.scalar_tensor_tensor(wc[:], wa[:], 1.0 / 16.0, wb[:],
                                        OP.mult, OP.bypass)
                nc.vector.copy_predicated(wind[:, e], msumf[:, e], wc[:])
            # token-major [p, e, f] -> gpsimd-wrapped [16, e, 256]: token t = f*128+p
            # sits at (g, s) = (p%16, 8f + p//16), so partition group b = p//16 maps
            # to the s%8 == b stripe -- 8 direct SBUF->SBUF DMAs, order-matched.
            indvw = rtt.tile([16, NE, 256], F32, tag="indvw", name="indvw")
            indww = rtt.tile([16, NE, 256], F32, tag="indww", name="indww")
            ivv = indvw[:].rearrange("g e (f b) -> g e f b", b=8)
            iwv = indww[:].rearrange("g e (f b) -> g e f b", b=8)
            for b in range(8):
                eng = (nc.scalar, nc.gpsimd)[b % 2]
                eng.dma_start(ivv[:, :, :, b], vind[16 * b:16 * (b + 1), :, :])
                eng.dma_start(iwv[:, :, :, b], wind[16 * b:16 * (b + 1), :, :])
            listv = rtt.tile([16, NE, CAP // 16], F32, tag="listv", name="listv")
            listw = rtt.tile([16, NE, CAP // 16], F32, tag="listw", name="listw")
            nft = rtt.tile([1, 2 * NE], U32, tag="nft", name="nft")
            for e in range(NE):
                nc.gpsimd.sparse_gather(listv[:, e], indvw[:, e],
                                        num_found=nft[:, e:e + 1])
                nc.gpsimd.sparse_gather(listw[:, e], indww[:, e],
                                        num_found=nft[:, NE + e:NE + e + 1])
            # clamp tails (-1 -> 0) and convert token lists to int16 idxs
            lvc = rtt.tile([16, 512], F32, tag="lvc", name="lvc")
            nc.vector.tensor_scalar_max(lvc[:], listv[:].rearrange("g e s -> g (e s)"), 0.0)
            lwc = rtt.tile([16, 512], F32, tag="lwc", name="lwc")
            nc.vector.tensor_scalar_max(lwc[:], listw[:].rearrange("g e s -> g (e s)"), 0.0)
            lvi = rtt.tile([16, 512], I16, tag="lvi", name="lvi")
            nc.vector.tensor_scalar_add(lvi[:], lvc[:], 0.0)
            for r in range(8):
                nc.scalar.dma_start(idxit[r * 16:(r + 1) * 16, :], lvi[:])
        # ---------- phase 5: gather-in + fp8 convert ----------
        with tc.tile_pool(name="gin", bufs=2) as gin:
            xbiF = xbi[:].rearrange("p t k -> p (t k)").bitcast(F32)
            xb2F = xb2[:].rearrange("p t k -> p (t k)").bitcast(F32)
            for h in range(4):
                hsl = slice(h * (S // 4), (h + 1) * (S // 4))
                isl = slice(h * 128, (h + 1) * 128)
                xgp = gin.tile([128, S // 4], F32, tag="xgp", name="xgp")
                nc.gpsimd.ap_gather(xgp[:], xbiF, idxit[:, isl], channels=128,
                                    num_elems=T, d=1, num_idxs=S // 4)
                xgpb = xgp[:].bitcast(BF16).rearrange("p (t k) -> p t k", k=2)
                nc.vector.tensor_scalar_add(xg8[:, 0, hsl], xgpb[:, :, 0], 0.0)
                nc.vector.tensor_scalar_add(xg8[:, 1, hsl], xgpb[:, :, 1], 0.0)
                xg2 = gin.tile([128, S // 4], F32, tag="xg2", name="xg2")
                nc.gpsimd.ap_gather(xg2[:], xb2F, idxit[:, isl], channels=128,
                                    num_elems=T, d=1, num_idxs=S // 4)
                xg2b = xg2[:].bitcast(BF16).rearrange("p (t k) -> p t k", k=2)
                nc.vector.tensor_scalar_add(xg8[:, 2, hsl], xg2b[:, :, 0], 0.0)

        # ---------- phase 4b: slot table + gather-out idxs (off the MLP path) ----------
        if True:
            # per-slot weights -> logical row -> broadcast to all partitions
            lwb = rtt.tile([16, 512], BF16, tag="lwb", name="lwb")
            nc.vector.tensor_scalar_add(lwb[:], lwc[:], 0.0)
            wrow = rtt.tile([1, S], BF16, tag="wrow", name="wrow")
            nc.sync.dma_start(w8dr[:], lwb[:])
            nc.sync.dma_start(wrow[:].rearrange("o (F g) -> o F g", g=16),
                              w8dr.rearrange("g F -> () F g"))
            nc.gpsimd.partition_broadcast(wslbc[:], wrow[:])

            # ranks: intra-tile prefix + inter-tile offsets per expert
            intra = rtt.tile([128, NE, 32], F32, tag="intra", name="intra")
            tTall = rtt.tile([32, 32], F32, tag="tTall", name="tTall")
            for e in range(NE):
                psI = rps.tile([128, 32], F32, tag="psI", name="psI")
                nc.tensor.matmul(psI[:], ltrit[:], mbf[:, e], start=True, stop=True)
                psT = rps.tile([128, 32], F32, tag="psT", name="psT")
                nc.tensor.matmul(psT[:], onest[:], mbf[:, e], start=True, stop=True)
                nc.vector.tensor_scalar_add(intra[:, e], psI[:], 0.0)
                tsb = rtt.tile([32, 32], F32, tag="tsb", name="tsb")
                nc.vector.tensor_scalar_add(tsb[:], psT[0:32, :], 0.0)
                tb32 = rtt.tile([32, 32], F32, tag="tb32", name="tb32")
                nc.vector.transpose(tb32[:], tsb[:])
                nc.vector.tensor_scalar_add(tTall[:, e:e + 1], tb32[:, 0:1], 0.0)
            tTbf = rtt.tile([32, NE], BF16, tag="tTbf", name="tTbf")
            nc.vector.tensor_scalar_add(tTbf[:], tTall[:, 0:NE], 0.0)
            psO = rps.tile([32, NE], F32, tag="psO", name="psO")
            nc.tensor.matmul(psO[:], ltrit[0:32, 0:32], tTbf[:], start=True, stop=True)
            oSb = rtt.tile([32, 32], F32, tag="oSb", name="oSb")
            nc.any.memset(oSb[:], 0.0)
            nc.vector.tensor_scalar_add(oSb[:, 0:NE], psO[:], 0.0)
            oT = rtt.tile([32, 32], F32, tag="oT", name="oT")
            nc.vector.transpose(oT[:], oSb[:])
            # bring the 8 expert offset rows to partition 0 (partition_broadcast
            # requires partition-0 input), then broadcast each
            offrow = rtt.tile([1, NE, 32], F32, tag="offrow", name="offrow")
            nc.sync.dma_start(odr[:], oT[0:NE, 0:32])
            nc.sync.dma_start(offrow[:], odr.rearrange("e f -> () e f"))
            offbc = rtt.tile([128, NE, 32], F32, tag="offbc", name="offbc")
            for e in range(NE):
                nc.gpsimd.partition_broadcast(offbc[:, e], offrow[0:1, e, :])
            # slots per token: slot_k = sum_e mk_e * min(intra+off, CAP-1) + e*CAP
            slot0 = rtt.tile([128, 32], F32, tag="slot0", name="slot0")
            slot1 = rtt.tile([128, 32], F32, tag="slot1", name="slot1")
            nc.any.memset(slot0[:], 0.0)
            nc.any.memset(slot1[:], 0.0)
            for e in range(NE):
                r1 = rtt.tile([128, 32], F32, tag="r1", name="r1")
                nc.vector.tensor_tensor(r1[:], intra[:, e], offbc[:, e], OP.add)
                nc.vector.tensor_scalar(r1[:], r1[:], float(CAP - 1), float(e * CAP),
                                        OP.min, OP.add)
                s0a = rtt.tile([128, 32], F32, tag="s0a", name="s0a")
                nc.vector.tensor_tensor(s0a[:], m0s[:, e], r1[:], OP.mult)
                nc.vector.tensor_tensor(slot0[:], slot0[:], s0a[:], OP.add)
                s1a = rtt.tile([128, 32], F32, tag="s1a", name="s1a")
                nc.vector.tensor_tensor(s1a[:], m1s[:, e], r1[:], OP.mult)
                nc.vector.tensor_tensor(slot1[:], slot1[:], s1a[:], OP.add)
            # interleaved (slot0,slot1) -> wrapped idx array [16, 512]
            nc.sync.dma_start(sl01[0].rearrange("(f p) -> p f", p=128), slot0[:])
            nc.sync.dma_start(sl01[1].rearrange("(f p) -> p f", p=128), slot1[:])
            idof = rtt.tile([16, 512], F32, tag="idof", name="idof")
            for k in range(2):
                nc.sync.dma_start(idof[k::2, :],
                                  sl01[k].rearrange("(f v) -> v f", v=8))
            ido16 = rtt.tile([16, 512], I16, tag="ido16", name="ido16")
            nc.vector.tensor_scalar_add(ido16[:], idof[:], 0.0)
            for r in range(8):
                nc.sync.dma_start(idxot[r * 16:(r + 1) * 16, :], ido16[:])

        rps.release()
        rtt.release()

        # ---------- phase 6: routed expert MLP over 16 blocks ----------
        xpool.release()  # xbi/xb2 dead once gather-in is done
        ypool = tc.alloc_tile_pool(name="ypool", bufs=1)
        yb01 = ypool.tile([128, S, 2], BF16, tag="yb01", name="yb01")
        yb2 = ypool.tile([128, S, 2], BF16, tag="yb2", name="yb2")
        with tc.tile_pool(name="wts", bufs=2) as wts, \
             tc.tile_pool(name="hsb", bufs=7) as hsb, \
             tc.tile_pool(name="hps", bufs=2, space="PSUM") as hps, \
             tc.tile_pool(name="yps", bufs=3, space="PSUM") as yps:
            for e in range(NE):
                w1t = wts.tile([128, 2, 2, HID], FP8, tag="w1t", name="w1t")
                nc.sync.dma_start(w1t[:], w1q[e].rearrange("a p j m -> p a j m"))
                w2pt = wts.tile([128, 6, 2, DIM], FP8, tag="w2pt", name="w2pt")
                nc.sync.dma_start(w2pt[:], w2p[e].rearrange("J p j m -> p J j m"))
                for bi in range(2):
                    blk = e * 2 + bi
                    sl = slice(blk * CB, (blk + 1) * CB)
                    hq8 = [hsb.tile([128, 2, CB], FP8, tag="hq8", name="hq8")
                           for _ in range(6)]
                    for J in range(6):
                        hp = hps.tile([128, 2 * CB], F32, tag="hp", name="hp")
                        for jj in range(2):
                            m = (2 * J + jj) * 128
                            nc.tensor.matmul(hp[:, jj * CB:(jj + 1) * CB],
                                             w1t[:, 0, :, m:m + 128],
                                             xg8[:, 0:2, sl], start=True, stop=False,
                                             perf_mode=mybir.MatmulPerfMode.DoubleRow)
                            nc.tensor.matmul(hp[:, jj * CB:(jj + 1) * CB],
                                             w1t[:, 1, :, m:m + 128],
                                             xg8[:, 2:4, sl], start=False, stop=True,
                                             perf_mode=mybir.MatmulPerfMode.DoubleRow)
                        nc.scalar.activation(hq8[J][:].rearrange("p j t -> p (j t)"),
                                             hp[:], AF.Gelu, bias=zerot[:],
                                             scale=1.0 / 16.0)
                    for dq in range(NQ):
                        py = yps.tile([128, CB], F32, tag="py", name="py")
                        for J in range(6):
                            nc.tensor.matmul(py[:],
                                             w2pt[:, J, :, dq * 128:(dq + 1) * 128],
                                             hq8[J][:],
                                             start=(J == 0), stop=(J == 5),
                                             perf_mode=mybir.MatmulPerfMode.DoubleRow)
                        ydst = (yb01[:, sl, dq] if dq < 2 else yb2[:, sl, 0])
                        nc.vector.scalar_tensor_tensor(
                            ydst, py[:], b2t[:, e, dq:dq + 1],
                            wslbc[:, sl], OP.add, OP.mult)

        # ---------- phase 7: gather-back + combine + residual ----------
        xg8pool.release()
        with tc.tile_pool(name="gout", bufs=1) as gout, \
             tc.tile_pool(name="fin", bufs=1) as fin:
            yb01F = yb01[:].rearrange("p s k -> p (s k)").bitcast(F32)
            yb2F = yb2[:].rearrange("p s k -> p (s k)").bitcast(F32)
            ygp = gout.tile([128, 2 * T], F32, tag="ygp", name="ygp")
            nc.gpsimd.ap_gather(ygp[:], yb01F, idxot[:], channels=128,
                                num_elems=S, d=1, num_idxs=2 * T)
            yg2 = gout.tile([128, 2 * T], F32, tag="yg2", name="yg2")
            nc.gpsimd.ap_gather(yg2[:], yb2F, idxot[:], channels=128,
                                num_elems=S, d=1, num_idxs=2 * T)
            ygpb = ygp[:].bitcast(BF16).rearrange("p (t k l) -> p t k l", k=2, l=2)
            yg2b = yg2[:].bitcast(BF16).rearrange("p (t k l) -> p t k l", k=2, l=2)
            for q in (0, 1, 2):
                ysrc = ygpb[:, :, :, q] if q < 2 else yg2b[:, :, :, 0]
                res = fin.tile([128, NIMG, 1024], F32, tag="res", name="res", bufs=2)
                nc.sync.dma_start(res[:], inp_cm[q * 128:(q + 1) * 128])
                t2 = fin.tile([128, T], BF16, tag="t2", name="t2")
                nc.vector.tensor_tensor(t2[:], ysrc[:, :, 0], ysrc[:, :, 1], OP.add)
                osb = fin.tile([128, NIMG, 1024], F32, tag="osb", name="osb")
                nc.vector.scalar_tensor_tensor(
                    osb.rearrange("p n x -> p (n x)"), t2[:],
                    chvt[:, q, 3:4], res.rearrange("p n x -> p (n x)"),
                    OP.mult, OP.add)
                nc.sync.dma_start(out_cm[q * 128:(q + 1) * 128], osb[:])

        ypool.release()
        dram.release()
        persist.release()

    nc.compile()
    return nc


def _prep(inputs):
    f8 = ml_dtypes.float8_e4m3
    bf = ml_dtypes.bfloat16
    dw_w = np.asarray(inputs["dw_w"], np.float32)  # [384,1,7,7]
    dgp = np.zeros((NQ, 7, 4, 128, 2, 128), np.float32)
    ii = np.arange(128)
    for q in range(NQ):
        for dw in range(7):
            for jp in range(3):
                for j in range(2):
                    dgp[q, dw, jp, ii, j, ii] = 16.0 * dw_w[q * 128:(q + 1) * 128, 0, 2 * jp + j, dw]
            dgp[q, dw, 3, ii, 0, ii] = 16.0 * dw_w[q * 128:(q + 1) * 128, 0, 6, dw]
    w1 = np.asarray(inputs["w1"], np.float32) * 16.0  # [8,384,1536]
    w2 = np.asarray(inputs["w2"], np.float32) * 16.0  # [8,1536,384]
    b1 = np.asarray(inputs["b1"], np.float32)  # [8,1536]
    w1q = np.zeros((NE, 2, 128, 2, HID), np.float32)
    w1q[:, 0] = w1[:, :256].reshape(NE, 2, 128, HID).transpose(0, 2, 1, 3)
    w1q[:, 1, :, 0, :] = w1[:, 256:]
    # bias fold: padded x chunk3 has 1/16 on partition 0; 256*b1 * (1/16) /16 = b1
    w1q[:, 1, 0, 1, :] = 256.0 * b1
    w2p = w2.reshape(NE, 6, 2, 128, DIM).transpose(0, 1, 3, 2, 4)
    b2 = np.asarray(inputs["b2"], np.float32)  # [8,384]
    b2s = 16.0 * b2.reshape(NE, NQ, 128).transpose(2, 0, 1)  # [128, 8, 3]
    gw = np.asarray(inputs["gate_w"], np.float32)  # [8,384]
    gws = gw.reshape(NE, NQ, 128).transpose(2, 1, 0)  # [128,3,8]
    chv = np.stack([
        np.asarray(inputs["dw_b"], np.float32),
        np.asarray(inputs["ln_g"], np.float32),
        np.asarray(inputs["ln_b"], np.float32),
        np.asarray(inputs["layer_scale"], np.float32).reshape(-1),
    ], axis=-1).reshape(NQ, 128, 4).transpose(1, 0, 2)  # [128,3,4]
    io8 = np.broadcast_to(np.arange(NE, dtype=np.float32), (128, NE))
    tid = (np.arange(32, dtype=np.float32)[None, :] * 128
           + np.arange(128, dtype=np.float32)[:, None])
    ltr = np.triu(np.ones((128, 128), np.float32), 1)  # ltr[p,m]=1 iff p<m
    common = {
        "dgp": np.ascontiguousarray(dgp.astype(f8)),
        "w1q": np.ascontiguousarray(w1q.astype(f8)),
        "w2p": np.ascontiguousarray(w2p.astype(f8)),
        "b2s": np.ascontiguousarray(b2s),
        "gws": np.ascontiguousarray(gws.astype(bf)),
        "chv": np.ascontiguousarray(chv),
        "io8": np.ascontiguousarray(io8),
        "tid": np.ascontiguousarray(tid),
        "ltr": np.ascontiguousarray(ltr.astype(bf)),
    }
    return common


def kernel(**inputs):
    global _cached
    if _cached is None:
        _cached = _build()
    nc = _cached
    common = _prep(inputs)
    inp = np.ascontiguousarray(np.asarray(inputs["input"], np.float32))
    f8 = ml_dtypes.float8_e4m3
    in_maps = []
    for c in range(8):
        m = dict(common)
        ci = inp[c * NIMG:(c + 1) * NIMG]
        m["inp4"] = np.ascontiguousarray(ci)
        pad = np.zeros((NQ, 128, 2, NIMG, 38, 38), np.float32)
        cc = ci.reshape(NIMG, NQ, 128, 32, 32).transpose(1, 2, 0, 3, 4)
        pad[:, :, 0, :, 3:35, 3:35] = cc
        pad[:, :, 1, :, 2:34, 3:35] = cc
        m["inp8"] = np.ascontiguousarray(pad.astype(f8))
        in_maps.append(m)
    res = bass_utils.run_bass_kernel_spmd(nc, in_maps, core_ids=list(range(8)))
    out = np.concatenate([res.results[c]["out4"] for c in range(8)], axis=0)
    return out.astype(np.float32)


if __name__ == "__main__":
    import reference
    inputs = {k: np.asarray(v) for k, v in reference.setup_inputs().items()}
    got = kernel(**inputs)
    exp = np.asarray(reference.reference(**reference.setup_inputs()))
    err = np.abs(got - exp)
    rel = err.max() / np.abs(exp).max()
    print("max abs err:", err.max(), "rel:", rel)
